# revision 20
# baseline (speedup 1.0000x reference)
"""Bidirectional Mamba block on 8 Trainium2 NeuronCores.

Sharding: core c in [0,8) handles batch b = c % 4, direction d = c // 4
(0=forward, 1=backward).  The host flips the sequence for backward cores,
pre-transposes weights into lhsT-ready layout, and sums the per-direction
partial projections (the all-reduce-at-out_proj unshard from the TP
formulation) plus residual on gather.

Device pipeline per core (single NeuronCore, fp32):
  LN -> transpose(xn) -> in_proj -> causal depthwise conv (4 diagonal
  matmuls PSUM-accumulated) -> SiLU -> x_proj -> dt_proj -> softplus ->
  selective scan: 16 states x 16 channel-tiles hardware tensor_tensor_scan
  with a = exp(A_n * delta) built by the ACT engine (per-partition scale),
  B/C broadcast via one-hot matmuls, n-reduction via identity-matmul PSUM
  accumulation -> D-skip + silu(z) gate -> out_proj -> final proj half
  with fused residual.
"""

import os
from contextlib import ExitStack
from dataclasses import dataclass

import numpy as np

import concourse.bass as bass
import concourse.tile as tile
from concourse import bacc, mybir
from concourse.bass_utils import run_bass_kernel_spmd

P = 128
FP32 = mybir.dt.float32
BF16 = mybir.dt.bfloat16
AF = mybir.ActivationFunctionType
ALU = mybir.AluOpType


@dataclass(frozen=True)
class Cfg:
    L: int = 2048        # sequence length
    DM: int = 1024       # d_model
    DI: int = 2048       # d_inner
    NS: int = 16         # d_state
    RK: int = 64         # dt_rank
    DC: int = 4          # d_conv
    TC: int = 256        # time chunk for the scan stage
    FT: int = 512        # matmul free-dim tile
    use_silu: bool = True  # False: x*sigmoid(x) fallback (CoreSim lacks Silu)
    scan_bf16: bool = True  # bf16 scan datapath (2x DVE/ACT modes)

    @property
    def nLt(self):  # L partition tiles
        return self.L // P

    @property
    def nMt(self):  # d_model partition tiles
        return self.DM // P

    @property
    def nDt(self):  # d_inner partition tiles
        return self.DI // P

    @property
    def nCh(self):  # scan chunks
        return self.L // self.TC

    @property
    def XD(self):   # x_dbl rows
        return self.RK + 2 * self.NS


CFG = Cfg()


def _onehot_banks(cfg: Cfg) -> tuple[np.ndarray, np.ndarray]:
    """lhsT banks that broadcast x_dbl row (RK+n) / (RK+NS+n) across all
    128 output partitions: out[m, t] = x_dbl[row, t] for m in [0,128)."""
    bb = np.zeros((cfg.XD, cfg.NS * P), np.float32)
    cb = np.zeros((cfg.XD, cfg.NS * P), np.float32)
    for n in range(cfg.NS):
        bb[cfg.RK + n, n * P:(n + 1) * P] = 1.0
        cb[cfg.RK + cfg.NS + n, n * P:(n + 1) * P] = 1.0
    return bb, cb


def build_program(cfg: Cfg = CFG):
    """Build the single-core SPMD Bass program. Returns (nc, input_names)."""
    nc = bacc.Bacc("TRN2", target_bir_lowering=False, debug=False)

    dp = nc.declare_dram_parameter
    x_in = dp("x_in", [cfg.L, cfg.DM], FP32, isOutput=False)
    w_in_T = dp("w_in_T", [2 * cfg.nDt, P, cfg.DM], FP32, isOutput=False)
    conv_w = dp("conv_w", [cfg.DI, cfg.DC], FP32, isOutput=False)
    conv_b = dp("conv_b", [cfg.DI], FP32, isOutput=False)
    w_xp_T = dp("w_xp_T", [cfg.DI, cfg.XD], FP32, isOutput=False)
    w_dtp_T = dp("w_dtp_T", [cfg.RK, cfg.DI], FP32, isOutput=False)
    dtp_b = dp("dtp_b", [cfg.DI], FP32, isOutput=False)
    a_mat = dp("a_mat", [cfg.DI, cfg.NS], FP32, isOutput=False)
    d_vec = dp("d_vec", [cfg.DI], FP32, isOutput=False)
    w_out_T = dp("w_out_T", [cfg.nMt, P, cfg.DI], FP32, isOutput=False)
    w_proj_T = dp("w_proj_T", [cfg.nMt, P, cfg.DM], FP32, isOutput=False)
    rbias = dp("rbias", [cfg.DM, cfg.L], FP32, isOutput=False)
    out_p = dp("out_p", [cfg.DM, cfg.L], FP32, isOutput=True)

    # DRAM scratch
    xc_dram = nc.dram_tensor("xc_scratch", [cfg.DI, cfg.L], FP32)
    sz_dram = nc.dram_tensor("sz_scratch", [cfg.DI, cfg.L], FP32)

    import ml_dtypes
    ident_np = np.eye(P, dtype=np.float32)
    bbank_np, cbank_np = _onehot_banks(cfg)
    ident_dram = nc.inline_tensor(ident_np, "ident_const")
    identbf_dram = nc.inline_tensor(
        np.eye(P).astype(ml_dtypes.bfloat16), "identbf_const")
    bbank_dram = nc.inline_tensor(bbank_np, "bbank_const")
    cbank_dram = nc.inline_tensor(cbank_np, "cbank_const")

    with tile.TileContext(nc) as tc, ExitStack() as ctx:
        _build_body(ctx, tc, cfg, dict(
            x_in=x_in.ap(), w_in_T=w_in_T.ap(), conv_w=conv_w.ap(),
            conv_b=conv_b.ap(), w_xp_T=w_xp_T.ap(), w_dtp_T=w_dtp_T.ap(),
            dtp_b=dtp_b.ap(), a_mat=a_mat.ap(), d_vec=d_vec.ap(),
            w_out_T=w_out_T.ap(), w_proj_T=w_proj_T.ap(), rbias=rbias.ap(),
            out_p=out_p.ap(), xc_dram=xc_dram.ap(), sz_dram=sz_dram.ap(),
            ident=ident_dram.ap(), bbank=bbank_dram.ap(), cbank=cbank_dram.ap(),
            identbf=identbf_dram.ap(),
        ))

    nc.compile()
    in_names = ["x_in", "w_in_T", "conv_w", "conv_b", "w_xp_T", "w_dtp_T",
                "dtp_b", "a_mat", "d_vec", "w_out_T", "w_proj_T", "rbias"]
    return nc, in_names


def _build_body(ctx: ExitStack, tc: tile.TileContext, cfg: Cfg, io: dict):
    nc = tc.nc
    L, DM, DI, NS, RK, DC, TC, FT = (cfg.L, cfg.DM, cfg.DI, cfg.NS, cfg.RK,
                                     cfg.DC, cfg.TC, cfg.FT)
    nLt, nMt, nDt, nCh, XD = cfg.nLt, cfg.nMt, cfg.nDt, cfg.nCh, cfg.XD
    nFt = L // FT           # full-length free tiles
    FTC = min(FT, TC)       # free tile within a chunk
    nFTC = TC // FTC

    consts = ctx.enter_context(tc.tile_pool(name="consts", bufs=1))
    wpool = ctx.enter_context(tc.tile_pool(name="wpool", bufs=2))

    # ---------------- constants to SBUF ----------------
    SDT = BF16 if cfg.scan_bf16 else FP32
    ident = consts.tile([P, P], FP32)
    nc.sync.dma_start(out=ident, in_=io["ident"])
    ident_acc = consts.tile([P, P], SDT)
    nc.sync.dma_start(out=ident_acc,
                      in_=io["identbf" if cfg.scan_bf16 else "ident"])
    bbank = consts.tile([XD, NS * P], FP32)
    nc.sync.dma_start(out=bbank, in_=io["bbank"])
    cbank = consts.tile([XD, NS * P], FP32)
    nc.sync.dma_start(out=cbank, in_=io["cbank"])

    a_sb = consts.tile([P, nDt, NS], FP32)
    nc.sync.dma_start(
        out=a_sb, in_=io["a_mat"].rearrange("(dt p) n -> p dt n", p=P))
    conv_w_sb = consts.tile([P, nDt, DC], FP32)
    nc.sync.dma_start(
        out=conv_w_sb, in_=io["conv_w"].rearrange("(dt p) j -> p dt j", p=P))
    conv_b_sb = consts.tile([P, nDt], FP32)
    nc.sync.dma_start(
        out=conv_b_sb, in_=io["conv_b"].rearrange("(dt p) -> p dt", p=P))
    dtp_b_sb = consts.tile([P, nDt], FP32)
    nc.sync.dma_start(
        out=dtp_b_sb, in_=io["dtp_b"].rearrange("(dt p) -> p dt", p=P))
    d_sb = consts.tile([P, nDt], FP32)
    nc.sync.dma_start(
        out=d_sb, in_=io["d_vec"].rearrange("(dt p) -> p dt", p=P))
    w_dtp_sb = consts.tile([RK, DI], FP32)
    nc.sync.dma_start(out=w_dtp_sb, in_=io["w_dtp_T"])
    w_xp_sb = consts.tile([P, nDt, XD], FP32)
    nc.sync.dma_start(
        out=w_xp_sb, in_=io["w_xp_T"].rearrange("(dt p) b -> p dt b", p=P))

    # carry state for chunked scans: column (dt*NS + n)
    carry = consts.tile([P, nDt * NS], FP32)
    nc.vector.memset(carry, 0.0)

    eps_t = consts.tile([P, 1], FP32)
    nc.vector.memset(eps_t, 1e-5)
    one_t = consts.tile([P, 1], FP32)
    nc.vector.memset(one_t, 1.0)

    xdbl = consts.tile([XD, L], FP32)

    # ---------------- stages 0+1: LN, transpose, in_proj, conv, x_dbl ------
    with tc.tile_pool(name="s01", bufs=1) as s01:
      xnT = s01.tile([P, nMt, L], FP32)
      with tc.tile_pool(name="ln_work", bufs=2) as work, \
           tc.tile_pool(name="ps_tr", bufs=2, space="PSUM") as ps_tr:
        inv_dm = 1.0 / DM
        for tt in range(nLt):
            xt = work.tile([P, DM], FP32, tag="ln_x")
            nc.sync.dma_start(out=xt, in_=io["x_in"][tt * P:(tt + 1) * P, :])
            ssum = work.tile([P, 1], FP32, tag="ln_s")
            nc.vector.tensor_reduce(ssum, xt, axis=mybir.AxisListType.X,
                                    op=ALU.add)
            mu = work.tile([P, 1], FP32, tag="ln_mu")
            nc.vector.tensor_scalar_mul(mu, ssum, inv_dm)
            xm = work.tile([P, DM], FP32, tag="ln_xm")
            nc.vector.tensor_scalar(xm, xt, mu, None, ALU.subtract)
            sq = work.tile([P, DM], FP32, tag="ln_x", name=f"sq{tt}")
            nc.scalar.square(sq, xm)
            ssq = work.tile([P, 1], FP32, tag="ln_ssq")
            nc.vector.tensor_reduce(ssq, sq, axis=mybir.AxisListType.X,
                                    op=ALU.add)
            std = work.tile([P, 1], FP32, tag="ln_std")
            nc.scalar.activation(std, ssq, AF.Sqrt, bias=eps_t, scale=inv_dm)
            rstd = work.tile([P, 1], FP32, tag="ln_rstd")
            nc.vector.reciprocal(rstd, std)
            xn = work.tile([P, DM], FP32, tag="ln_xn")
            nc.vector.tensor_scalar_mul(xn, xm, rstd)
            for k in range(nMt):
                pt = ps_tr.tile([P, P], FP32, tag="tr")
                nc.tensor.transpose(pt, xn[:, k * P:(k + 1) * P], ident)
                nc.scalar.copy(xnT[:, k, tt * P:(tt + 1) * P], pt)

      with tc.tile_pool(name="s01_big", bufs=2) as s01b, \
           tc.tile_pool(name="work", bufs=2) as work, \
           tc.tile_pool(name="ps_mm", bufs=2, space="PSUM") as ps_mm, \
           tc.tile_pool(name="ps_xdbl", bufs=1, space="PSUM") as ps_xdbl:
        # x_dbl accumulates in PSUM across the whole xc-half m-loop.
        xdbl_ps = [ps_xdbl.tile([XD, FT], FP32, tag=f"xdblp{f}",
                                name=f"xdbl_ps{f}") for f in range(nFt)]

        for m in range(nDt):  # xc half of in_proj
            wm = wpool.tile([P, nMt, P], FP32, tag="w_in")
            nc.sync.dma_start(out=wm, in_=io["w_in_T"][m])
            xcr = s01b.tile([P, DC - 1 + L], FP32, tag="xcraw")
            nc.vector.memset(xcr[:, 0:DC - 1], 0.0)
            for f in range(nFt):
                pp = ps_mm.tile([P, FT], FP32, tag="mm")
                for k in range(nMt):
                    nc.tensor.matmul(pp, wm[:, k, :],
                                     xnT[:, k, f * FT:(f + 1) * FT],
                                     start=(k == 0), stop=(k == nMt - 1))
                nc.vector.tensor_copy(
                    xcr[:, DC - 1 + f * FT:DC - 1 + (f + 1) * FT], pp)
            # depthwise causal conv as DC diagonal matmuls + silu
            dg = work.tile([P, DC, P], FP32, tag="diag")
            for j in range(DC):
                nc.vector.tensor_scalar_mul(dg[:, j, :], ident,
                                            conv_w_sb[:, m, j:j + 1])
            xc_m = s01b.tile([P, L], FP32, tag="xc_m")
            for f in range(nFt):
                pc = ps_mm.tile([P, FT], FP32, tag="cv")
                for j in range(DC):
                    nc.tensor.matmul(pc, dg[:, j, :],
                                     xcr[:, j + f * FT:j + f * FT + FT],
                                     start=(j == 0), stop=(j == DC - 1))
                if cfg.use_silu:
                    nc.scalar.activation(xc_m[:, f * FT:(f + 1) * FT], pc,
                                         AF.Silu, bias=conv_b_sb[:, m:m + 1])
                else:
                    tv = work.tile([P, FT], FP32, tag="silu_t")
                    nc.scalar.activation(tv, pc, AF.Identity,
                                         bias=conv_b_sb[:, m:m + 1])
                    sg = work.tile([P, FT], FP32, tag="silu_s")
                    nc.scalar.activation(sg, tv, AF.Sigmoid)
                    nc.vector.tensor_mul(xc_m[:, f * FT:(f + 1) * FT], tv, sg)
            nc.sync.dma_start(out=io["xc_dram"][m * P:(m + 1) * P, :],
                              in_=xc_m)
            # x_dbl accumulation: xdbl += w_xp[m].T @ xc_m
            for f in range(nFt):
                nc.tensor.matmul(xdbl_ps[f], w_xp_sb[:, m, :],
                                 xc_m[:, f * FT:(f + 1) * FT],
                                 start=(m == 0), stop=(m == nDt - 1))

        for f in range(nFt):
            nc.vector.tensor_copy(xdbl[:, f * FT:(f + 1) * FT], xdbl_ps[f])

        for m in range(nDt):  # z half of in_proj -> silu -> spill
            wm = wpool.tile([P, nMt, P], FP32, tag="w_in")
            nc.sync.dma_start(out=wm, in_=io["w_in_T"][nDt + m])
            sz_m = s01b.tile([P, L], FP32, tag="sz_m")
            for f in range(nFt):
                pp = ps_mm.tile([P, FT], FP32, tag="mm")
                for k in range(nMt):
                    nc.tensor.matmul(pp, wm[:, k, :],
                                     xnT[:, k, f * FT:(f + 1) * FT],
                                     start=(k == 0), stop=(k == nMt - 1))
                if cfg.use_silu:
                    nc.scalar.activation(sz_m[:, f * FT:(f + 1) * FT], pp,
                                         AF.Silu)
                else:
                    tv = work.tile([P, FT], FP32, tag="silu_t")
                    nc.scalar.copy(tv, pp)
                    sg = work.tile([P, FT], FP32, tag="silu_s")
                    nc.scalar.activation(sg, tv, AF.Sigmoid)
                    nc.vector.tensor_mul(sz_m[:, f * FT:(f + 1) * FT], tv, sg)
            nc.sync.dma_start(out=io["sz_dram"][m * P:(m + 1) * P, :],
                              in_=sz_m)

    # ---------------- stage 2: chunked selective scan ----------------
    with tc.tile_pool(name="bcpool", bufs=1) as bcpool, \
         tc.tile_pool(name="hpool", bufs=2) as hpool, \
         tc.tile_pool(name="bpool", bufs=2) as bpool, \
         tc.tile_pool(name="ygpool", bufs=1) as ygpool, \
         tc.tile_pool(name="yopool", bufs=1) as yopool, \
         tc.tile_pool(name="scanp", bufs=4) as scanp, \
         tc.tile_pool(name="work2", bufs=3) as work, \
         tc.tile_pool(name="ps_bc", bufs=2, space="PSUM") as ps_bc, \
         tc.tile_pool(name="ps_dlt", bufs=2, space="PSUM") as ps_dlt, \
         tc.tile_pool(name="ps_yacc", bufs=2, space="PSUM") as ps_yacc, \
         tc.tile_pool(name="ps_out", bufs=1, space="PSUM") as ps_out:
        # n-indices whose hC runs on gpsimd (DVE keeps the rest)
        HC_GPS = set(range(NS - 6, NS))
        for ci in range(nCh):
            t0 = ci * TC
            # B/C broadcast tiles: out[m, t] = x_dbl[RK(+NS)+n, t] for all m
            bbc = bcpool.tile([P, NS, TC], SDT, tag="bbc", name=f"bbc{ci}")
            cbc = bcpool.tile([P, NS, TC], SDT, tag="cbc", name=f"cbc{ci}")
            for n in range(NS):
                for fc in range(nFTC):
                    pb = ps_bc.tile([P, FTC], FP32, tag="bc")
                    nc.tensor.matmul(pb, bbank[:, n * P:(n + 1) * P],
                                     xdbl[:, t0 + fc * FTC:t0 + (fc + 1) * FTC],
                                     start=True, stop=True)
                    nc.scalar.copy(bbc[:, n, fc * FTC:(fc + 1) * FTC], pb)
                    pcs = ps_bc.tile([P, FTC], FP32, tag="bc")
                    nc.tensor.matmul(pcs, cbank[:, n * P:(n + 1) * P],
                                     xdbl[:, t0 + fc * FTC:t0 + (fc + 1) * FTC],
                                     start=True, stop=True)
                    nc.scalar.copy(cbc[:, n, fc * FTC:(fc + 1) * FTC], pcs)

            yg = ygpool.tile([P, nDt, TC], FP32, tag="yg", name=f"yg{ci}")
            for dt in range(nDt):
                # delta = softplus(dt_proj + bias)
                pd = ps_dlt.tile([P, TC], FP32, tag="dlt")
                for fc in range(nFTC):
                    nc.tensor.matmul(
                        pd[:, fc * FTC:(fc + 1) * FTC],
                        w_dtp_sb[:, dt * P:(dt + 1) * P],
                        xdbl[0:RK, t0 + fc * FTC:t0 + (fc + 1) * FTC],
                        start=True, stop=True)
                # softplus(x+b) = ln(1 + exp(x+b)); x+b is O(1) so exp is safe
                edz = work.tile([P, TC], FP32, tag="edz")
                nc.scalar.activation(edz, pd, AF.Exp,
                                     bias=dtp_b_sb[:, dt:dt + 1])
                delta = work.tile([P, TC], FP32, tag="delta")
                nc.scalar.activation(delta, edz, AF.Ln, bias=one_t)
                xc_c = work.tile([P, TC], FP32, tag="xc_c")
                nc.sync.dma_start(
                    out=xc_c,
                    in_=io["xc_dram"][dt * P:(dt + 1) * P, t0:t0 + TC])
                du = work.tile([P, TC], SDT, tag="du")
                nc.gpsimd.tensor_mul(du, delta, xc_c)

                # all b inputs up front (gpsimd), so the DVE scan stream and
                # the gpsimd hC work never block each other
                b_all = bpool.tile([P, NS, TC], SDT, tag="b",
                                   name=f"b_{ci}_{dt}")
                for n in range(NS):
                    nc.gpsimd.tensor_mul(b_all[:, n, :], du, bbc[:, n, :])
                h_dt = hpool.tile([P, NS, TC], SDT, tag="h",
                                  name=f"h_{ci}_{dt}")
                yp = ps_yacc.tile([P, TC], FP32, tag="yacc")
                for n in range(NS):
                    a_t = scanp.tile([P, TC], SDT, tag="a")
                    nc.scalar.activation(a_t, delta, AF.Exp,
                                         scale=a_sb[:, dt, n:n + 1])
                    col = dt * NS + n
                    nc.vector.tensor_tensor_scan(
                        h_dt[:, n, :], a_t, b_all[:, n, :],
                        carry[:, col:col + 1], ALU.mult, ALU.add)
                    hc = scanp.tile([P, TC], SDT, tag="hc")
                    heng = nc.gpsimd if n in HC_GPS else nc.vector
                    heng.tensor_mul(hc, h_dt[:, n, :], cbc[:, n, :])
                    for fc in range(nFTC):
                        nc.tensor.matmul(yp[:, fc * FTC:(fc + 1) * FTC],
                                         ident_acc,
                                         hc[:, fc * FTC:(fc + 1) * FTC],
                                         start=(n == 0), stop=(n == NS - 1))
                # save carries for next chunk: h[:, :, -1] -> carry cols
                nc.vector.tensor_copy(carry[:, dt * NS:(dt + 1) * NS],
                                      h_dt[:, :, TC - 1:TC])
                # gate: (y + xc*D) * silu(z)
                sz_c = work.tile([P, TC], FP32, tag="sz_c")
                nc.sync.dma_start(
                    out=sz_c,
                    in_=io["sz_dram"][dt * P:(dt + 1) * P, t0:t0 + TC])
                t1 = work.tile([P, TC], FP32, tag="gate1")
                nc.vector.scalar_tensor_tensor(
                    t1, in0=xc_c, scalar=d_sb[:, dt:dt + 1], in1=yp,
                    op0=ALU.mult, op1=ALU.add)
                nc.vector.tensor_mul(yg[:, dt, :], t1, sz_c)

            # out_proj for this chunk
            yo = yopool.tile([P, nMt, TC], FP32, tag="yo", name=f"yo{ci}")
            for m in range(nMt):
                wo = wpool.tile([P, nDt, P], FP32, tag="w_out")
                nc.sync.dma_start(out=wo, in_=io["w_out_T"][m])
                po = ps_out.tile([P, TC], FP32, tag="oproj")
                for fc in range(nFTC):
                    for dt in range(nDt):
                        nc.tensor.matmul(po[:, fc * FTC:(fc + 1) * FTC],
                                         wo[:, dt, :],
                                         yg[:, dt, fc * FTC:(fc + 1) * FTC],
                                         start=(dt == 0), stop=(dt == nDt - 1))
                nc.scalar.copy(yo[:, m, :], po)

            # final proj half + residual
            for m in range(nMt):
                wp = wpool.tile([P, nMt, P], FP32, tag="w_proj")
                nc.sync.dma_start(out=wp, in_=io["w_proj_T"][m])
                pf = ps_out.tile([P, TC], FP32, tag="fproj")
                for fc in range(nFTC):
                    for k in range(nMt):
                        nc.tensor.matmul(pf[:, fc * FTC:(fc + 1) * FTC],
                                         wp[:, k, :],
                                         yo[:, k, fc * FTC:(fc + 1) * FTC],
                                         start=(k == 0), stop=(k == nMt - 1))
                rb = work.tile([P, TC], FP32, tag="rb")
                nc.sync.dma_start(
                    out=rb, in_=io["rbias"][m * P:(m + 1) * P, t0:t0 + TC])
                pout = work.tile([P, TC], FP32, tag="pout")
                nc.vector.tensor_add(pout, pf, rb)
                nc.sync.dma_start(
                    out=io["out_p"][m * P:(m + 1) * P, t0:t0 + TC], in_=pout)


# ---------------------------------------------------------------------------
# Host-side wrapper
# ---------------------------------------------------------------------------

_PROGRAM_CACHE: dict = {}


def _get_program(cfg: Cfg = CFG):
    key = cfg
    if key not in _PROGRAM_CACHE:
        _PROGRAM_CACHE[key] = build_program(cfg)
    return _PROGRAM_CACHE[key]


def _f32(a) -> np.ndarray:
    return np.ascontiguousarray(np.asarray(a, dtype=np.float32))


def _cast_w(a: np.ndarray, cfg: Cfg) -> np.ndarray:
    if not cfg.scan_bf16:
        return a
    import ml_dtypes
    return np.ascontiguousarray(a.astype(ml_dtypes.bfloat16))


def prep_core_inputs(c: int, inputs: dict, cfg: Cfg = CFG) -> dict:
    B = inputs["x"].shape[0]
    b, d = c % B, c // B
    sfx = "f" if d == 0 else "b"
    x = _f32(inputs["x"])
    xb = x[b] if d == 0 else x[b][::-1]
    proj_w = _f32(inputs["proj_w"])
    proj_b = _f32(inputs["proj_b"])
    if d == 0:
        rb = np.ascontiguousarray(xb.T + proj_b[:, None])
    else:
        rb = np.zeros((cfg.DM, cfg.L), np.float32)
    def pack(wT, nblk):
        # [K, M] -> [M/P slabs, P, K] with slab m rows = K-partition-tiled
        Kd, Md = wT.shape
        nk = Kd // P
        return np.ascontiguousarray(
            wT.reshape(nk, P, Md // P, P).transpose(2, 1, 0, 3)
              .reshape(Md // P, P, Kd))

    return {
        "x_in": np.ascontiguousarray(xb),
        "w_in_T": pack(np.ascontiguousarray(_f32(inputs[f"in_w_{sfx}"]).T),
                       2 * cfg.nDt),
        "conv_w": np.ascontiguousarray(_f32(inputs[f"conv_w_{sfx}"])[:, 0, :]),
        "conv_b": _f32(inputs[f"conv_b_{sfx}"]),
        "w_xp_T": np.ascontiguousarray(_f32(inputs[f"xp_w_{sfx}"]).T),
        "w_dtp_T": np.ascontiguousarray(_f32(inputs[f"dtp_w_{sfx}"]).T),
        "dtp_b": _f32(inputs[f"dtp_b_{sfx}"]),
        "a_mat": np.ascontiguousarray(-np.exp(_f32(inputs[f"A_log_{sfx}"]))),
        "d_vec": _f32(inputs[f"D_{sfx}"]),
        "w_out_T": pack(np.ascontiguousarray(_f32(inputs[f"out_w_{sfx}"]).T),
                        cfg.nMt),
        "w_proj_T": pack(np.ascontiguousarray(
            proj_w[:, d * cfg.DM:(d + 1) * cfg.DM].T), cfg.nMt),
        "rbias": rb,
    }


LAST_RESULT = None


def kernel(**inputs) -> np.ndarray:
    global LAST_RESULT
    cfg = CFG
    B = inputs["x"].shape[0]
    n_cores = 2 * B
    nc, in_names = _get_program(cfg)
    in_maps = [prep_core_inputs(c, inputs, cfg) for c in range(n_cores)]
    trace = bool(os.environ.get("BASS_TRACE"))
    res = run_bass_kernel_spmd(nc, in_maps, list(range(n_cores)), trace=trace)
    LAST_RESULT = res
    out = np.empty((B, cfg.L, cfg.DM), np.float32)
    for b in range(B):
        pf = res.results[b]["out_p"].T
        pb = res.results[B + b]["out_p"].T[::-1]
        out[b] = pf + pb
    return out
